# revision 1
# baseline (speedup 1.0000x reference)
"""Euclidean-distance attention on 8 Trainium2 NeuronCores.

Sharding: batch (2) x head-groups (4 heads each) -> 8 cores; each core
computes Q/K/V projections for its 4 heads (column-sliced weights), a
flash-style transposed-score attention, and a partial out-projection
(row-sliced wo). Host sums the 4 partials per batch (row-parallel out_proj
reduction) and adds the output bias.

Math trick: softmax_k(-max(||q||^2+||k||^2-2qk, 0)/T) == softmax_k((2qk-||k||^2)/T)
(the ||q||^2 term is constant per row and cancels; the max() clamp never fires
because d^2 >= 0 up to rounding).  With scores computed transposed
(scT[k, q] = K @ Q^T), the per-k bias -||k||^2/T is a per-partition vector and
folds into the scalar-engine exp activation: p~ = exp(scale*scT + bias).
Normalization uses an extra all-ones column appended to V, so the softmax
denominator falls out of the same PSUM accumulation as the numerator.
"""

import sys

sys.path.insert(0, "/opt/trn_rl_repo")

import numpy as np

import concourse.bass as bass
import concourse.tile as tile
from concourse import bacc, mybir
from concourse.bass_utils import run_bass_kernel_spmd
from concourse.masks import make_identity

F32 = mybir.dt.float32
BF16 = mybir.dt.bfloat16

E = 1024          # embed dim
D = 64            # head dim
HLOC = 4          # heads per core
DH = HLOC * D     # 256: per-core projection width
P = 128
N_CORES = 8


def build_program(S, temperature, zq, zk, zv):
    """Trace the per-core program. All 8 cores run this same program on
    different input slices. zq/zk/zv: bias-is-zero flags (skip the adds)."""
    T = float(temperature)
    NT = S // P           # token tiles
    NE = E // P           # embed (contraction) tiles
    NPR = HLOC // 2       # head pairs (2)
    QW = min(512, S)      # q block width for score matmuls
    NQB = S // QW         # q blocks
    GW = 2 * QW           # exp tile width (2 q-blocks share one ACT call)

    # Bacc (not raw Bass): its compile() passes legalize multi-wait
    # instructions (move_matmul_waits_to_ldweights, generate_event_semaphores)
    # for the 1-wait-per-instruction hardware encoding.
    nc = bacc.Bacc(None)
    x_d = nc.dram_tensor("x", [S, E], F32, kind="ExternalInput")
    wq_d = nc.dram_tensor("wq_s", [E, DH], F32, kind="ExternalInput")
    wk_d = nc.dram_tensor("wk_s", [E, DH], F32, kind="ExternalInput")
    wv_d = nc.dram_tensor("wv_s", [E, DH], F32, kind="ExternalInput")
    wo_d = nc.dram_tensor("wo_s", [DH, E], F32, kind="ExternalInput")
    bq_d = nc.dram_tensor("bq_s", [DH], F32, kind="ExternalInput")
    bk_d = nc.dram_tensor("bk_s", [DH], F32, kind="ExternalInput")
    bv_d = nc.dram_tensor("bv_s", [DH], F32, kind="ExternalInput")
    # one output tensor per token tile, stored straight from PSUM: 16
    # stores spread across the 16 DMA queues (8 HWDGE + 8 SWDGE) so each
    # queue sees one store -> no same-queue ordering wait, leaving the
    # single descriptor wait slot for the RAW wait on the matmuls
    y_ds = [
        nc.dram_tensor(f"y{tt}", [P, E], F32, kind="ExternalOutput")
        for tt in range(S // P)
    ]

    def bcast_ap(ap_1d, parts):
        # [N] dram vector -> [parts, N] partition-broadcast AP
        return bass.AP(
            tensor=ap_1d.tensor, offset=ap_1d.offset, ap=[[0, parts]] + list(ap_1d.ap)
        )

    with tile.TileContext(nc) as tc:
        with tc.tile_pool(name="consts", bufs=1) as consts, \
             tc.tile_pool(name="big", bufs=1) as big, \
             tc.tile_pool(name="xbpool", bufs=3) as xbpool, \
             tc.tile_pool(name="pTpool", bufs=3) as pTpool, \
             tc.tile_pool(name="dbpool", bufs=4) as dbpool, \
             tc.tile_pool(name="xstage", bufs=1) as xstage:
            # ---- constants / weights staging ----
            ident = consts.tile([P, P], BF16)
            make_identity(nc, ident)

            # weights: casting SWDGE DMAs (f32->bf16 in flight), chunked
            # per contraction tile so the 8 SWDGE queues run in parallel
            wq_sb = consts.tile([P, NE, DH], BF16)
            wk_sb = consts.tile([P, NE, DH], BF16)
            wv_sb = consts.tile([P, NE, DH], BF16)
            wo_sb = consts.tile([P, 2, E], BF16)

            # all-ones stationary for the denominator broadcast matmul;
            # row 64 (= base_partition of the denominator row) is what's used
            ones_col = consts.tile([P, D], F32)
            nc.vector.memset(ones_col, 1.0)

            bq_col = consts.tile([P, NPR], F32)
            nc.gpsimd.dma_start(bq_col, bq_d[:].rearrange("(pr p) -> p pr", p=P))
            bk_col = consts.tile([P, NPR], F32)
            nc.gpsimd.dma_start(bk_col, bk_d[:].rearrange("(pr p) -> p pr", p=P))
            bk_bc = consts.tile([P, DH], F32)
            nc.gpsimd.dma_start(bk_bc, bcast_ap(bk_d[:], P))
            bv_bc = consts.tile([P, DH], F32)
            nc.gpsimd.dma_start(bv_bc, bcast_ap(bv_d[:], P))

            # ---- persistent big tiles ----
            xb_all = big.tile([P, NT, E], BF16)    # query, cast to bf16
            qT = big.tile([P, NE, S], BF16)        # query^T (e-major)
            QT_sb = big.tile([P, NPR, S], BF16)    # Q^T per head-pair
            KT_sb = big.tile([P, NPR, S], BF16)
            V_sb = big.tile([P, NT, HLOC, D + 1], BF16)   # V + ones column
            nksq = big.tile([P, NT, HLOC], F32)    # -||k||^2 / T
            ou_all = big.tile([P, HLOC, NQB, QW], F32)  # unnormalized attn out
            aoT = big.tile([P, NPR, S], BF16)      # normalized attn out^T

            nc.gpsimd.memset(V_sb[:, :, :, D], 1.0)

            # first x tiles via HWDGE f32 loads + DVE casts: both are much
            # lower-latency than casting SWDGE DMAs, and the first
            # transposes gate the whole projection phase
            for j in range(QW // P):
                xs = xstage.tile([P, E], F32, tag=f"xs{j}", name=f"xs{j}")
                nc.sync.dma_start(xs, x_d[j * P:(j + 1) * P, :])
                nc.vector.tensor_copy(xb_all[:, j, :], xs)
            for w_d, w_sb in ((wq_d, wq_sb), (wk_d, wk_sb), (wv_d, wv_sb)):
                wr = w_d[:].rearrange("(e p) d -> p e d", p=P)
                for e in range(NE):
                    nc.gpsimd.dma_start(w_sb[:, e, :], wr[:, e, :])
            wor = wo_d[:].rearrange("(s p) d -> p s d", p=P)
            for s2 in range(2):
                nc.gpsimd.dma_start(wo_sb[:, s2, :], wor[:, s2, :])

            # ---- phase 1+2: transpose x, projections ----
            with tc.tile_pool(name="ps_tr", bufs=2, space="PSUM") as ps_tr, \
                 tc.tile_pool(name="ps_pj", bufs=2, space="PSUM") as ps_pj, \
                 tc.tile_pool(name="ps_kv", bufs=2, space="PSUM") as ps_kv:
                for blk in range(S // QW):
                    jlo = blk * (QW // P)
                    jhi = jlo + QW // P
                    for j in range(jlo, jhi):
                        if j >= QW // P:  # first block's DMAs issued above
                            nc.gpsimd.dma_start(
                                xb_all[:, j, :], x_d[j * P:(j + 1) * P, :])
                        for e in range(NE):
                            pt = ps_tr.tile([P, P], BF16, tag="tr")
                            nc.tensor.transpose(pt, xb_all[:, j, e * P:(e + 1) * P], ident)
                            nc.vector.tensor_copy(qT[:, e, j * P:(j + 1) * P], pt)
                    bsl = slice(blk * QW, (blk + 1) * QW)
                    # Q^T and K^T per head pair over this token block
                    for pr in range(NPR):
                        psl = slice(pr * P, (pr + 1) * P)
                        for w_sb, dst, bias_col, bz in (
                            (wq_sb, QT_sb, bq_col[:, pr:pr + 1], zq),
                            (wk_sb, KT_sb, bk_col[:, pr:pr + 1], zk),
                        ):
                            pj = ps_pj.tile([P, QW], F32, tag="pj")
                            for e in range(NE):
                                nc.tensor.matmul(
                                    pj,
                                    lhsT=w_sb[:, e, psl],
                                    rhs=qT[:, e, bsl],
                                    start=(e == 0),
                                    stop=(e == NE - 1),
                                )
                            if bz:
                                nc.vector.tensor_copy(dst[:, pr, bsl], pj)
                            else:
                                # bias is per-partition (head-dim) in ^T layout
                                nc.vector.tensor_scalar_add(
                                    out=dst[:, pr, bsl], in0=pj, scalar1=bias_col
                                )
                    # V (token-major) and -||k||^2/T over this token block
                    for j in range(jlo, jhi):
                        pv = ps_kv.tile([P, DH], F32, tag="pv")
                        for e in range(NE):
                            nc.tensor.matmul(
                                pv,
                                lhsT=qT[:, e, j * P:(j + 1) * P],
                                rhs=wv_sb[:, e, :],
                                start=(e == 0),
                                stop=(e == NE - 1),
                            )
                        for h in range(HLOC):
                            if zv:
                                # scalar engine, so the attn*V matmul's waits
                                # on V and on the exp output share one sem
                                nc.scalar.copy(
                                    V_sb[:, j, h, 0:D], pv[:, h * D:(h + 1) * D]
                                )
                            else:
                                nc.vector.tensor_add(
                                    out=V_sb[:, j, h, 0:D],
                                    in0=pv[:, h * D:(h + 1) * D],
                                    in1=bv_bc[:, h * D:(h + 1) * D],
                                )
                        pk = ps_kv.tile([P, DH], F32, tag="pk")
                        for e in range(NE):
                            nc.tensor.matmul(
                                pk,
                                lhsT=qT[:, e, j * P:(j + 1) * P],
                                rhs=wk_sb[:, e, :],
                                start=(e == 0),
                                stop=(e == NE - 1),
                            )
                        sq_t = xbpool.tile([P, DH], F32, tag="sq")
                        if zk:
                            # only one non-scalar PSUM input allowed per DVE op
                            nc.vector.tensor_copy(sq_t, pk)
                        else:
                            nc.vector.tensor_add(out=sq_t, in0=pk, in1=bk_bc)
                        nc.vector.tensor_mul(sq_t, sq_t, sq_t)
                        ksqr = xbpool.tile([P, HLOC], F32, tag="ksqr")
                        nc.vector.tensor_reduce(
                            out=ksqr,
                            in_=sq_t.rearrange("p (h d) -> p h d", h=HLOC),
                            axis=mybir.AxisListType.X,
                            op=mybir.AluOpType.add,
                        )
                        nc.scalar.mul(nksq[:, j, :], ksqr, -1.0 / T)

            # ---- phase 3: attention, q-block-pair outer ----
            # Processing q-block pairs outermost lets each pair's
            # normalization + out-projection overlap the next pair's
            # attention instead of sitting exposed at the kernel tail.
            # PSUM: scores 2x2 banks + attn*V 2 banks + tail bc/py 2 = 8.
            with tc.tile_pool(name="ps_sc", bufs=2, space="PSUM") as ps_sc, \
                 tc.tile_pool(name="ps_av", bufs=1, space="PSUM") as ps_av, \
                 tc.tile_pool(name="ps_tl", bufs=1, space="PSUM") as ps_tl, \
                 tc.tile_pool(name="ypool", bufs=4) as ypool:
                for g0 in range(0, NQB, 2):
                    gn = min(2, NQB - g0)
                    for h in range(HLOC):
                        pr = h // 2
                        off = (h % 2) * D
                        av_ts = [
                            ps_av.tile([P, QW], F32, tag=f"av{qq}",
                                       name=f"av{g0}_{h}_{qq}")
                            for qq in range(gn)
                        ]
                        if g0 == 0 and h == 0:
                            for qq in range(gn):
                                nc.vector.memset(av_ts[qq], 0.0)
                        for j in range(NT):
                            sc_t = ps_sc.tile([P, gn * QW], F32, tag="sc")
                            for qq in range(gn):
                                qb = g0 + qq
                                nc.tensor.matmul(
                                    sc_t[:, qq * QW:(qq + 1) * QW],
                                    lhsT=KT_sb[off:off + D, pr, j * P:(j + 1) * P],
                                    rhs=QT_sb[off:off + D, pr, qb * QW:(qb + 1) * QW],
                                    start=True,
                                    stop=True,
                                )
                            pT_t = pTpool.tile([P, GW], BF16, tag="pT")
                            nc.scalar.activation(
                                out=pT_t[:, :gn * QW],
                                in_=sc_t,
                                func=mybir.ActivationFunctionType.Exp,
                                bias=nksq[:, j, h:h + 1],
                                scale=2.0 / T,
                            )
                            for qq in range(gn):
                                nc.tensor.matmul(
                                    av_ts[qq][: D + 1, :],
                                    lhsT=V_sb[:, j, h, :],
                                    rhs=pT_t[:, qq * QW:(qq + 1) * QW],
                                    start=(j == 0),
                                    stop=(j == NT - 1),
                                )
                        # drain unnormalized outputs; frees the av banks for
                        # the next head while normalization runs elsewhere
                        for qq in range(gn):
                            nc.vector.tensor_copy(
                                ou_all[: D + 1, h, g0 + qq, :],
                                av_ts[qq][: D + 1, :],
                            )
                    # normalization + out-projection for this q-block pair;
                    # overlaps the next pair's attention (disjoint data,
                    # dedicated ps_tl banks)
                    last = (g0 + 2 >= NQB)
                    nbc = 0
                    for qq in range(gn):
                        qb = g0 + qq
                        for h in range(HLOC):
                            pr = h // 2
                            off = (h % 2) * D
                            # for the final pair attention is done, so the
                            # idle attn*V banks deepen the tail pipeline
                            if last and nbc % 2 == 1:
                                bc_t = ps_av.tile([P, QW], F32, tag="av0",
                                                  name="bcb")[:, :GW // 2]
                            else:
                                bc_t = ps_tl.tile([P, GW], F32, tag="tl",
                                                  name="bc")
                            nbc += 1
                            nc.tensor.matmul(
                                bc_t[:D, :QW],
                                lhsT=ones_col[D:D + 1, :],
                                rhs=ou_all[D:D + 1, h, qb, :],
                                start=True,
                                stop=True,
                            )
                            rb_t = dbpool.tile([D, QW], F32, tag="rb")
                            nc.vector.reciprocal(rb_t, bc_t[:D, :QW])
                            nc.vector.tensor_mul(
                                aoT[off:off + D, pr, qb * QW:(qb + 1) * QW],
                                ou_all[:D, h, qb, :],
                                rb_t,
                            )
                        for tt in range(4 * qb, 4 * qb + 4):
                            # final pair: rotate py across the idle score
                            # slots too (2x depth on the exposed tail)
                            if last and tt % 2 == 1:
                                py = ps_sc.tile([P, GW], F32, tag="sc",
                                                name="pyb")
                            else:
                                py = ps_tl.tile([P, GW], F32, tag="tl",
                                                name="py")
                            for oh in range(E // 512):
                                for s in range(2):
                                    nc.tensor.matmul(
                                        py[:, oh * 512:(oh + 1) * 512],
                                        lhsT=aoT[:, s, tt * P:(tt + 1) * P],
                                        rhs=wo_sb[:, s, oh * 512:(oh + 1) * 512],
                                        start=(s == 0),
                                        stop=(s == 1),
                                    )
                            y_t = ypool.tile([P, E], F32, tag="y")
                            nc.scalar.copy(y_t, py)
                            eng = nc.sync if tt % 2 == 0 else nc.gpsimd
                            eng.dma_start(y_ds[tt][:, :], y_t)

    # run Bacc's compile passes (wait legalization, register allocation);
    # run_bass_via_pjrt binds the module without finalizing it
    nc.finalize()
    return nc


def make_in_maps(inputs, S):
    q = np.ascontiguousarray(np.asarray(inputs["query"], np.float32))
    wq = np.asarray(inputs["wq"], np.float32)
    wk = np.asarray(inputs["wk"], np.float32)
    wv = np.asarray(inputs["wv"], np.float32)
    wo = np.asarray(inputs["wo"], np.float32)
    bq = np.asarray(inputs["bq"], np.float32)
    bk = np.asarray(inputs["bk"], np.float32)
    bv = np.asarray(inputs["bv"], np.float32)
    in_maps = []
    for c in range(N_CORES):
        b = c // 4
        lo = (c % 4) * DH
        in_maps.append({
            "x": np.ascontiguousarray(q[b, :S]),
            "wq_s": np.ascontiguousarray(wq[:, lo:lo + DH]),
            "wk_s": np.ascontiguousarray(wk[:, lo:lo + DH]),
            "wv_s": np.ascontiguousarray(wv[:, lo:lo + DH]),
            "wo_s": np.ascontiguousarray(wo[lo:lo + DH, :]),
            "bq_s": np.ascontiguousarray(bq[lo:lo + DH]),
            "bk_s": np.ascontiguousarray(bk[lo:lo + DH]),
            "bv_s": np.ascontiguousarray(bv[lo:lo + DH]),
        })
    return in_maps


_prog_cache = {}


def _get_program(S, T, zq, zk, zv):
    key = (S, T, zq, zk, zv)
    if key not in _prog_cache:
        _prog_cache[key] = build_program(S, T, zq, zk, zv)
    return _prog_cache[key]


def _run(inputs, trace=False, tmpdir=None):
    S = np.asarray(inputs["query"]).shape[1]
    T = float(np.asarray(inputs["temperature"]))
    zq = not np.any(np.asarray(inputs["bq"]))
    zk = not np.any(np.asarray(inputs["bk"]))
    zv = not np.any(np.asarray(inputs["bv"]))
    nc = _get_program(S, T, zq, zk, zv)
    in_maps = make_in_maps(inputs, S)
    res = run_bass_kernel_spmd(
        nc, in_maps, list(range(N_CORES)), trace=trace, tmpdir=tmpdir
    )
    ng = S // 128
    ys = [
        np.concatenate([res.results[i][f"y{g}"] for g in range(ng)], axis=0)
        for i in range(N_CORES)
    ]
    bo = np.asarray(inputs["bo"], np.float32)
    out = np.stack([
        ys[0] + ys[1] + ys[2] + ys[3],
        ys[4] + ys[5] + ys[6] + ys[7],
    ]).astype(np.float32)
    out += bo[None, None, :]
    return out, res


def kernel(**inputs):
    out, _ = _run(inputs, trace=False)
    return out



# revision 2
# speedup vs baseline: 1.0168x; 1.0168x over previous
"""Euclidean-distance attention on 8 Trainium2 NeuronCores (v2).

Sharding: batch (2) x head-groups (4 heads each) -> 8 cores; each core
computes Q/K/V projections for its 4 heads (column-sliced weights), a
flash-style transposed-score attention, and a partial out-projection
(row-sliced wo). Host sums the 4 partials per batch and adds the bias.

v2 math: softmax_k(-d^2/T) @ V == (sum_k e^{2qk/T} V'[k]) / (sum_k e^{2qk/T} c[k])
with c[k] = exp(-||k||^2/T) and V' = V * c.  Folding the -||k||^2/T term
into V (instead of the per-partition exp bias) makes the exp bias a
CONSTANT, so one scalar-engine activation covers both heads of a pair
(1024 columns per call) -- the scalar engine is the phase-2 bottleneck
and per-call overhead is ~250ns.  ||k||^2 comes from squaring K^T on the
vector engine and a block-diagonal ones matmul (partition reduction per
head), replacing the v1 token-major K projection (-27k PE cycles).

Phase 2 runs pair-outer; each (pair, q-group) tail (normalize + out-proj)
is issued as morsels interleaved into the next q-group's k-loop so
neither PE nor ACT idles long enough to re-throttle the HAM clock.
"""

import sys

sys.path.insert(0, "/opt/trn_rl_repo")

import numpy as np

import concourse.bass as bass
import concourse.tile as tile
from concourse import bacc, mybir
from concourse.bass_utils import run_bass_kernel_spmd
from concourse.masks import make_identity

F32 = mybir.dt.float32
BF16 = mybir.dt.bfloat16

E = 1024          # embed dim
D = 64            # head dim
HLOC = 4          # heads per core
NPR = 2           # head pairs per core
DH = HLOC * D     # 256: per-core projection width
P = 128
N_CORES = 8


def build_program(S, temperature, zq, zk, zv):
    T = float(temperature)
    NT = S // P           # token tiles (16)
    NE = E // P           # embed (contraction) tiles (8)
    QW = 512              # q columns per attention step
    NQG = S // QW         # q groups (4)
    BW = 512              # phase-1 token block width
    NB = S // BW          # blocks (4)
    JPB = BW // P         # token tiles per block (4)
    RC = 4                # reciprocal chunks per head

    nc = bacc.Bacc(None)
    xT_d = nc.dram_tensor("xT", [E, S], F32, kind="ExternalInput")
    wq_d = nc.dram_tensor("wq_s", [E, DH], F32, kind="ExternalInput")
    wk_d = nc.dram_tensor("wk_s", [E, DH], F32, kind="ExternalInput")
    wv_d = nc.dram_tensor("wv_s", [E, DH], F32, kind="ExternalInput")
    wo_d = nc.dram_tensor("wo_s", [DH, E], F32, kind="ExternalInput")
    bq_d = nc.dram_tensor("bq_s", [DH], F32, kind="ExternalInput")
    bk_d = nc.dram_tensor("bk_s", [DH], F32, kind="ExternalInput")
    bv_d = nc.dram_tensor("bv_s", [DH], F32, kind="ExternalInput")
    y_ds = [
        nc.dram_tensor(f"y{tt}", [P, E], BF16, kind="ExternalOutput")
        for tt in range(NT)
    ]

    def bcast_ap(ap_1d, parts):
        return bass.AP(
            tensor=ap_1d.tensor, offset=ap_1d.offset, ap=[[0, parts]] + list(ap_1d.ap)
        )

    with tile.TileContext(nc) as tc:
        with tc.tile_pool(name="consts", bufs=1) as consts, \
             tc.tile_pool(name="big", bufs=1) as big, \
             tc.tile_pool(name="sqpool", bufs=2) as sqpool, \
             tc.tile_pool(name="kqpool", bufs=2) as kqpool, \
             tc.tile_pool(name="pTpool", bufs=3) as pTpool, \
             tc.tile_pool(name="oupool", bufs=2) as oupool, \
             tc.tile_pool(name="denpool", bufs=2) as denpool, \
             tc.tile_pool(name="ypool", bufs=4) as ypool, \
             tc.tile_pool(name="xstage", bufs=1) as xstage:
            ident = consts.tile([P, P], BF16)
            make_identity(nc, ident)

            wq_sb = consts.tile([P, NE, DH], BF16)
            wk_sb = consts.tile([P, NE, DH], BF16)
            wv_sb = consts.tile([P, NE, DH], BF16)
            wo_sb = consts.tile([P, 2, E], BF16)

            # rank-1 broadcast source for the denominator (bf16: an f32
            # ones_col would lower the bc matmul to a 4-pass fp32 LOW_HIGH
            # matmul, ~4us of PE at every q-group boundary), and the
            # block-diagonal ones used to reduce (K^T)^2 over d per head
            identf2 = consts.tile([2, 2], F32)
            nc.vector.tensor_copy(identf2, ident[0:2, 0:2])
            onesblk = consts.tile([P, NPR], BF16)
            nc.vector.memset(onesblk, 0.0)
            nc.vector.memset(onesblk[0:D, 0:1], 1.0)
            nc.vector.memset(onesblk[D:P, 1:2], 1.0)

            bq_col = consts.tile([P, NPR], F32)
            nc.gpsimd.dma_start(bq_col, bq_d[:].rearrange("(pr p) -> p pr", p=P))
            bk_col = consts.tile([P, NPR], F32)
            nc.gpsimd.dma_start(bk_col, bk_d[:].rearrange("(pr p) -> p pr", p=P))
            bv_bc = consts.tile([P, DH], F32)
            if not zv:
                nc.gpsimd.dma_start(bv_bc, bcast_ap(bv_d[:], P))

            # ---- persistent big tiles ----
            qT = big.tile([P, NE, S], BF16)         # x^T (e-major)
            QT_sb = big.tile([P, NPR, S], BF16)     # Q^T per head pair
            KT_sb = big.tile([P, NPR, S], BF16)
            V_sb = big.tile([P, NT, HLOC, D + 1], BF16)  # V' + ecol column
            ecol_f = big.tile([P, NT, HLOC], F32)   # exp(-||k||^2/T)
            aoT = big.tile([P, NPR, S], BF16)       # normalized attn out^T
            pT0 = big.tile([P, NT, 2 * QW], BF16)   # probs of (pair0, qg0)

            # first block of x^T: low-latency HWDGE f32 loads + DVE casts
            # (x^T comes pre-transposed from the host -- sharding prep --
            # which removes all 128 PE-mode transposes from the kernel)
            for e in range(NE):
                xs = xstage.tile([P, BW], F32, tag=f"xs{e}", name=f"xs{e}")
                nc.sync.dma_start(xs, xT_d[e * P:(e + 1) * P, 0:BW])
                nc.vector.tensor_copy(qT[:, e, 0:BW], xs)
            # one big casting descriptor per weight tensor / x^T chunk:
            # the gpsimd DMA *issue* instructions cost ~0.7us each, so 58
            # small descriptors serialized phase 1 on the issue stream
            for w_d, w_sb in ((wk_d, wk_sb), (wq_d, wq_sb), (wv_d, wv_sb)):
                wr = w_d[:].rearrange("(e p) d -> p e d", p=P)
                for h2 in range(2):
                    esl = slice(h2 * (NE // 2), (h2 + 1) * (NE // 2))
                    nc.gpsimd.dma_start(w_sb[:, esl, :], wr[:, esl, :])
            for e in range(NE):
                nc.gpsimd.dma_start(
                    qT[:, e, BW:S], xT_d[e * P:(e + 1) * P, BW:S])
            wor = wo_d[:].rearrange("(s p) d -> p s d", p=P)
            nc.gpsimd.dma_start(wo_sb[:, :, :], wor[:, :, :])

            # ---- phase 1: transpose x, projections, ||k||^2 -> ecol, V' ----
            with tc.tile_pool(name="ps_pj", bufs=2, space="PSUM") as ps_pj, \
                 tc.tile_pool(name="ps_pv", bufs=1, space="PSUM") as ps_pv, \
                 tc.tile_pool(name="ps_kq", bufs=1, space="PSUM") as ps_kq, \
                 tc.tile_pool(name="ps_ke", bufs=1, space="PSUM") as ps_ke, \
                 tc.tile_pool(name="ps_sc0", bufs=1, space="PSUM") as ps_sc0:

                def ladder_sc0(jlo2, jhi2):
                    # scores+exp of (pair 0, q-group 0) interleaved into the
                    # phase-1 block stream: the scalar engine would otherwise
                    # idle until all projections finish.  Single-buffered sc
                    # (PSUM is tight in phase 1); probs park in pT0 and the
                    # attn*V accumulation catches up after phase 1.
                    for j2 in range(jlo2, jhi2):
                        sc = ps_sc0.tile([P, 2 * QW], F32, tag="sc0")
                        for hin in range(2):
                            dsl = slice(hin * D, (hin + 1) * D)
                            nc.tensor.matmul(
                                sc[:, hin * QW:(hin + 1) * QW],
                                lhsT=KT_sb[dsl, 0, j2 * P:(j2 + 1) * P],
                                rhs=QT_sb[dsl, 0, 0:QW],
                                start=True, stop=True,
                            )
                        nc.scalar.activation(
                            out=pT0[:, j2, :], in_=sc,
                            func=mybir.ActivationFunctionType.Exp,
                            scale=2.0 / T,
                        )

                for blk in range(NB):
                    jlo = blk * JPB
                    bsl = slice(blk * BW, (blk + 1) * BW)
                    # K^T per pair (+ squared K^T on DVE as it lands)
                    kecolps = ps_ke.tile([P, JPB, HLOC], BF16, tag="ke")
                    sq_tiles = {}
                    for pr in range(NPR):
                        pj = ps_pj.tile([P, BW], F32, tag="pj")
                        for e in range(NE):
                            nc.tensor.matmul(
                                pj,
                                lhsT=wk_sb[:, e, pr * P:(pr + 1) * P],
                                rhs=qT[:, e, bsl],
                                start=(e == 0),
                                stop=(e == NE - 1),
                            )
                        if zk:
                            nc.scalar.copy(KT_sb[:, pr, bsl], pj)
                        else:
                            nc.vector.tensor_scalar_add(
                                out=KT_sb[:, pr, bsl], in0=pj,
                                scalar1=bk_col[:, pr:pr + 1])
                        sq_t = sqpool.tile([P, BW], BF16, tag=f"sq{pr}")
                        nc.vector.tensor_mul(
                            sq_t, KT_sb[:, pr, bsl], KT_sb[:, pr, bsl])
                        sq_tiles[pr] = sq_t
                    # Q^T per pair (PE streams on while the DVE chain runs)
                    for pr in range(NPR):
                        pj = ps_pj.tile([P, BW], F32, tag="pj")
                        for e in range(NE):
                            nc.tensor.matmul(
                                pj,
                                lhsT=wq_sb[:, e, pr * P:(pr + 1) * P],
                                rhs=qT[:, e, bsl],
                                start=(e == 0),
                                stop=(e == NE - 1),
                            )
                        if zq:
                            nc.scalar.copy(QT_sb[:, pr, bsl], pj)
                        else:
                            nc.vector.tensor_scalar_add(
                                out=QT_sb[:, pr, bsl], in0=pj,
                                scalar1=bq_col[:, pr:pr + 1])
                        # ||k||^2 reduction for this pair, issued late so the
                        # transposes never wait on the DVE square/copy
                        ksqps = ps_kq.tile([2, BW], F32, tag="kq")
                        nc.tensor.matmul(
                            ksqps, lhsT=onesblk, rhs=sq_tiles[pr],
                            start=True, stop=True)
                        ksq_sb = kqpool.tile([2, BW], BF16, tag="ksq")
                        nc.vector.tensor_copy(ksq_sb, ksqps)
                        for c in range(JPB):
                            nc.tensor.transpose(
                                kecolps[:, c, pr * 2:pr * 2 + 2],
                                ksq_sb[0:2, c * P:(c + 1) * P],
                                ident[0:2, 0:2],
                            )
                    # exp(-||k||^2/T) for the whole block in one ACT call
                    ecv = ecol_f[:, jlo:jlo + JPB, :]
                    nc.scalar.activation(
                        out=ecv, in_=kecolps,
                        func=mybir.ActivationFunctionType.Exp,
                        scale=-1.0 / T,
                    )
                    nc.vector.tensor_copy(V_sb[:, jlo:jlo + JPB, :, D], ecv)
                    ladder_sc0(blk * JPB, (blk + 1) * JPB)
                    # V' = (V + bv) * ecol, token-major; one psum tile for
                    # the whole block so the MMs never wait on the drains
                    pv = ps_pv.tile([P, JPB, DH], F32, tag="pv")
                    for jj in range(JPB):
                        j = jlo + jj
                        for e in range(NE):
                            nc.tensor.matmul(
                                pv[:, jj, :],
                                lhsT=qT[:, e, j * P:(j + 1) * P],
                                rhs=wv_sb[:, e, :],
                                start=(e == 0),
                                stop=(e == NE - 1),
                            )
                    for jj in range(JPB):
                        j = jlo + jj
                        for h in range(HLOC):
                            if zv:
                                nc.vector.tensor_scalar_mul(
                                    out=V_sb[:, j, h, 0:D],
                                    in0=pv[:, jj, h * D:(h + 1) * D],
                                    scalar1=ecol_f[:, j, h:h + 1],
                                )
                            else:
                                vtmp = sqpool.tile([P, D], F32, tag="vtmp")
                                nc.vector.tensor_add(
                                    out=vtmp, in0=pv[:, jj, h * D:(h + 1) * D],
                                    in1=bv_bc[:, h * D:(h + 1) * D])
                                nc.vector.tensor_scalar_mul(
                                    out=V_sb[:, j, h, 0:D], in0=vtmp,
                                    scalar1=ecol_f[:, j, h:h + 1],
                                )

            # ---- phase 2: flash attention, pair-outer ----
            # PSUM: scores dbuf 2x[128,1024] = 4 banks, attn*V accum 2 banks,
            # tail (bc/py) 2 banks = 8.  Tail work for (pr, qg) is issued as
            # morsels inside the NEXT q-group's k-loop so PE/ACT stay hot.
            with tc.tile_pool(name="ps_sc", bufs=2, space="PSUM") as ps_sc, \
                 tc.tile_pool(name="ps_av", bufs=1, space="PSUM") as ps_av, \
                 tc.tile_pool(name="ps_tl", bufs=1, space="PSUM") as ps_tl:

                def tail_drain(ts):
                    """Drain prev av accumulators to SBUF (before av realloc)."""
                    pr, qg = ts["pr"], ts["qg"]
                    ts["ou"] = []
                    ts["den"] = []
                    for hin in range(2):
                        ou = oupool.tile([P, QW], F32, tag=f"ou{hin}",
                                         name=f"ou{pr}_{qg}_{hin}")
                        nc.vector.tensor_copy(ou[:D, :], ts["av"][hin][:D, :])
                        den = denpool.tile([1, QW], F32, tag=f"den{hin}",
                                           name=f"den{pr}_{qg}_{hin}")
                        nc.vector.tensor_copy(den, ts["av"][hin][D:D + 1, :])
                        ts["ou"].append(ou)
                        ts["den"].append(den)

                def norm_morsel(m, ts):
                    """Normalization morsels of tail ts, one window after
                    its q-group.  The DVE reciprocal costs ~6.5ns/elem, so
                    instead of 2x[64,512] reciprocals (~7us serial) the two
                    denominator rows are PE-transposed into a [128,8] block,
                    reciprocal'd in one ~0.2us call, transposed back, and
                    the *reciprocal* rows are rank-1 broadcast (bf16 matmul)
                    for the normalize multiplies to read from PSUM."""
                    pr, qg = ts["pr"], ts["qg"]
                    qsl = slice(qg * QW, (qg + 1) * QW)
                    if m == 1:
                        tl = ps_tl.tile([P, 2 * QW], F32, tag="tl",
                                        name=f"tl{pr}_{qg}")
                        ts["tl"] = tl
                        for c in range(4):
                            for hin in range(2):
                                nc.tensor.transpose(
                                    tl[:, c * 2 + hin:c * 2 + hin + 1],
                                    ts["den"][hin][0:1, c * P:(c + 1) * P],
                                    identf2[0:1, 0:1],
                                )
                        return
                    if m == 2:
                        rbs = denpool.tile([P, 8], BF16, tag="rbs",
                                           name=f"rbs{pr}_{qg}")
                        ts["rbs"] = rbs
                        with nc.allow_low_precision(
                                reason="1/denominator in bf16 (~0.4%) is "
                                "within the softmax tolerance"):
                            nc.vector.reciprocal(rbs, ts["tl"][:, 0:8])
                        return
                    if m in (3, 4):
                        # broadcast recip rows back over 64 partitions: a
                        # stride-0 free-dim lhsT against the identity gives
                        # out[m, q] = rbs[q, col] for all m
                        hin = m - 3
                        for c in range(4):
                            col = ts["rbs"][:, 2 * c + hin:2 * c + hin + 1]
                            lhsT_b = bass.AP(
                                tensor=col.tensor, offset=col.offset,
                                ap=[col.ap[0], [0, D]])
                            nc.tensor.matmul(
                                ts["tl"][:D,
                                         hin * QW + c * P:hin * QW + (c + 1) * P],
                                lhsT=lhsT_b, rhs=ident[:, :],
                                start=True, stop=True,
                            )
                        return
                    if m in (5, 6):
                        hin = m - 5
                        nc.vector.tensor_mul(
                            aoT[hin * D:(hin + 1) * D, pr, qsl],
                            ts["ou"][hin][:D, :],
                            ts["tl"][:D, hin * QW:(hin + 1) * QW])

                def proj_morsel(m, ts, py_pool=None, yt_eng=None):
                    """Out-projection morsels of tail ts, two windows after
                    its q-group (aoT is ready by then, so the matmuls never
                    block the PE queue)."""
                    pr, qg = ts["pr"], ts["qg"]
                    if pr == 1 and m in (8, 10, 12, 14):
                        c = (m - 8) // 2
                        tt = qg * 4 + c
                        pool = py_pool if py_pool is not None else ps_tl
                        tag = "sc" if py_pool is not None else "tl"
                        py = pool.tile([P, 2 * QW], F32, tag=tag,
                                       name=f"py{qg}_{tt}")
                        yt = ypool.tile([P, E], BF16, tag="y", name=f"y{tt}")
                        for oh in range(2):
                            for s in range(2):
                                nc.tensor.matmul(
                                    py[:, oh * QW:(oh + 1) * QW],
                                    lhsT=aoT[:, s, tt * P:(tt + 1) * P],
                                    rhs=wo_sb[:, s, oh * QW:(oh + 1) * QW],
                                    start=(s == 0), stop=(s == 1),
                                )
                        if yt_eng == "scalar":
                            nc.scalar.copy(yt, py)
                        else:
                            nc.vector.tensor_copy(yt, py)
                        eng = nc.sync if tt % 2 == 0 else nc.gpsimd
                        eng.dma_start(y_ds[tt][:, :], yt)

                prevN = None  # tail awaiting normalization (window N+1)
                prevP = None  # tail awaiting out-projection (window N+2)
                # catch-up: attn*V of (pair 0, q-group 0), whose scores/exps
                # ran laddered inside phase 1 with probs parked in pT0
                av0 = [
                    ps_av.tile([P, QW], F32, tag=f"av{hin}", name=f"av0_{hin}")
                    for hin in range(2)
                ]
                for j in range(NT):
                    for hin in range(2):
                        nc.tensor.matmul(
                            av0[hin][:D + 1, :],
                            lhsT=V_sb[:, j, hin, :],
                            rhs=pT0[:, j, hin * QW:(hin + 1) * QW],
                            start=(j == 0),
                            stop=(j == NT - 1),
                        )
                prevN = {"pr": 0, "qg": 0, "av": av0}
                for pr in range(NPR):
                    for qg in range(NQG):
                        if pr == 0 and qg == 0:
                            continue
                        qsl = slice(qg * QW, (qg + 1) * QW)
                        if prevN is not None:
                            tail_drain(prevN)
                        av = [
                            ps_av.tile([P, QW], F32, tag=f"av{hin}",
                                       name=f"av{pr}_{qg}_{hin}")
                            for hin in range(2)
                        ]
                        for j in range(NT):
                            sc = ps_sc.tile([P, 2 * QW], F32, tag="sc")
                            for hin in range(2):
                                dsl = slice(hin * D, (hin + 1) * D)
                                nc.tensor.matmul(
                                    sc[:, hin * QW:(hin + 1) * QW],
                                    lhsT=KT_sb[dsl, pr, j * P:(j + 1) * P],
                                    rhs=QT_sb[dsl, pr, qsl],
                                    start=True, stop=True,
                                )
                            pT = pTpool.tile([P, 2 * QW], BF16, tag="pT")
                            nc.scalar.activation(
                                out=pT, in_=sc,
                                func=mybir.ActivationFunctionType.Exp,
                                scale=2.0 / T,
                            )
                            for hin in range(2):
                                h = pr * 2 + hin
                                nc.tensor.matmul(
                                    av[hin][:D + 1, :],
                                    lhsT=V_sb[:, j, h, :],
                                    rhs=pT[:, hin * QW:(hin + 1) * QW],
                                    start=(j == 0),
                                    stop=(j == NT - 1),
                                )
                            if prevN is not None:
                                norm_morsel(j, prevN)
                            if prevP is not None:
                                proj_morsel(j, prevP)
                        prevP = prevN
                        prevN = {"pr": pr, "qg": qg, "av": av}
                # final tails, exposed: prevN's norm chain first (it is
                # short now), prevP's out-proj concurrently; py buffers
                # ping-pong between the tl slot and a freed scores slot,
                # and half the yt copies go to the idle scalar engine
                tail_drain(prevN)
                for m in range(1, 8):
                    norm_morsel(m, prevN)
                for k, ts in ((0, prevP), (1, prevN)):
                    for c in range(4):
                        proj_morsel(8 + 2 * c, ts,
                                    py_pool=(ps_sc if c % 2 else None),
                                    yt_eng=("scalar" if c % 2 else None))

    nc.finalize()
    return nc


def make_in_maps(inputs, S):
    q = np.ascontiguousarray(np.asarray(inputs["query"], np.float32))
    wq = np.asarray(inputs["wq"], np.float32)
    wk = np.asarray(inputs["wk"], np.float32)
    wv = np.asarray(inputs["wv"], np.float32)
    wo = np.asarray(inputs["wo"], np.float32)
    bq = np.asarray(inputs["bq"], np.float32)
    bk = np.asarray(inputs["bk"], np.float32)
    bv = np.asarray(inputs["bv"], np.float32)
    in_maps = []
    for c in range(N_CORES):
        b = c // 4
        lo = (c % 4) * DH
        in_maps.append({
            "xT": np.ascontiguousarray(q[b, :S].T),
            "wq_s": np.ascontiguousarray(wq[:, lo:lo + DH]),
            "wk_s": np.ascontiguousarray(wk[:, lo:lo + DH]),
            "wv_s": np.ascontiguousarray(wv[:, lo:lo + DH]),
            "wo_s": np.ascontiguousarray(wo[lo:lo + DH, :]),
            "bq_s": np.ascontiguousarray(bq[lo:lo + DH]),
            "bk_s": np.ascontiguousarray(bk[lo:lo + DH]),
            "bv_s": np.ascontiguousarray(bv[lo:lo + DH]),
        })
    return in_maps


_prog_cache = {}


def _get_program(S, T, zq, zk, zv):
    key = (S, T, zq, zk, zv)
    if key not in _prog_cache:
        _prog_cache[key] = build_program(S, T, zq, zk, zv)
    return _prog_cache[key]


def _run(inputs, trace=False, tmpdir=None):
    S = np.asarray(inputs["query"]).shape[1]
    T = float(np.asarray(inputs["temperature"]))
    zq = not np.any(np.asarray(inputs["bq"]))
    zk = not np.any(np.asarray(inputs["bk"]))
    zv = not np.any(np.asarray(inputs["bv"]))
    nc = _get_program(S, T, zq, zk, zv)
    in_maps = make_in_maps(inputs, S)
    res = run_bass_kernel_spmd(
        nc, in_maps, list(range(N_CORES)), trace=trace, tmpdir=tmpdir
    )
    ng = S // 128
    ys = [
        np.concatenate(
            [np.asarray(res.results[i][f"y{g}"], np.float32) for g in range(ng)],
            axis=0,
        )
        for i in range(N_CORES)
    ]
    bo = np.asarray(inputs["bo"], np.float32)
    out = np.stack([
        ys[0] + ys[1] + ys[2] + ys[3],
        ys[4] + ys[5] + ys[6] + ys[7],
    ]).astype(np.float32)
    out += bo[None, None, :]
    return out, res


def kernel(**inputs):
    out, _ = _run(inputs, trace=False)
    return out
